# revision 3
# baseline (speedup 1.0000x reference)
"""MultiHeadDenseAttention on 8 Trainium2 NeuronCores.

Head-sharded tensor parallelism: each core computes 2 of 16 heads
(value projection slice, per-head MLP attention logits, softmax, S@V),
then an AllToAll exchanges head-blocks for row-blocks so each core
computes the output projection for its 512 rows with the full Wo.

v2 vs v1: mixed-precision row-count reduction on the PE.
  - logits matmul in fp8e4m3 DoubleRow mode (2 k-tiles of 64 contracted
    per pass, 0.5 cycles/row): hid stored fp8 [64, 2, 4096] with k-tile 1
    holding the ones row (fused b2) + zero padding; W2 host-scaled by 8
    (fp8 dynamic range) and un-scaled in the exp activation (scale=1/8).
  - value projection computed directly in [m, d] layout (lhsT = x chunk,
    rhs = Wv.T block, PSUM accumulation over the 8 feature chunks):
    kills the 32 fp32 PE transposes and the SBUF re-accumulation pass.
  - value/S@V/output-projection in bf16 (same PE rate as fp32r at these
    sizes, half the SBUF/DMA bytes); x streamed as bf16 (8.4 MB/rep).

Layouts (per core c, heads 2c / 2c+1):
  xt    [1024, 4096] bf16  x.reshape(4096,1024).T  (feat on partitions)
  xc    [128, 4096] f32r   xt rows [128c, 128c+128) (this core's head cols)
  wv    [128, 1024] bf16   col block f = Wv.T[128f:+128, 128c:+128]
  vh[b] [128, 16, 130] bf16  per m-chunk: 65 (h0 dims+one) + 65 (h1)
  hidT[h] [64, 2, 4096] fp8  k-tile 0 = relu(W1@x), k-tile 1 = ones/zeros
  w2dr  [64, 2, 2048] fp8  k-tile 0 = 8*W2.T, k-tile 1 row 0 = 8*b2
  logitsT/expT [128m, 512n] per m-chunk (softmax w/o max: logits O(1))
  S@V: po[65, 512] = vh_aug.T @ expT  (row 64 = sum of exp)
  A2A [8, 2, 65, 512] f32, normalize after exchange, out = act @ Wo.T.
"""

import sys

if "/opt/trn_rl_repo" not in sys.path:
    sys.path.insert(0, "/opt/trn_rl_repo")

from contextlib import ExitStack

import numpy as np
import ml_dtypes

import bass_rust
import concourse.bass as bass
import concourse.tile as tile
from concourse import mybir
from concourse.bass_utils import run_bass_kernel_spmd

F32 = mybir.dt.float32
F32R = mybir.dt.float32r
BF16 = mybir.dt.bfloat16
F8 = mybir.dt.float8e4
AF = mybir.ActivationFunctionType

NC = 8            # cores
B = 2             # batch
N_SEQ = 2048      # seq len == max_seq_len (m)
FEAT = 1024
H = 16            # heads
D = 64            # head dim
NTOT = B * N_SEQ  # 4096 flattened rows
NBLK = 512        # n-block size
NB = NTOT // NBLK # 8 n-blocks (== A2A shards == cores)
MC = N_SEQ // 128 # 16 m-chunks per batch
W2SCALE = 8.0     # host-side W2 scaling into fp8 range; undone in exp


def _split_sem_waits(nc, limit=1):
    """Walrus rejects instructions with more than ~1 sync wait; move the
    excess onto NOPs on the same engine inserted immediately before."""
    blocks = {}
    for f in nc.m.functions:
        for bb in f.blocks:
            blocks[bb.name] = bb
    for bb in blocks.values():
        i = 0
        while i < len(bb.instructions):
            inst = bb.instructions[i]
            si = inst.sync_info
            if si is not None and si.on_wait and len(si.on_wait) > limit:
                waits = list(si.on_wait)
                chunks = [waits[j : j + limit] for j in range(0, len(waits), limit)]
                si.on_wait = chunks[-1]
                engine = nc.engines[inst.engine]
                for chunk in chunks[:-1]:
                    d = engine.nop(nofuse=True, hint="wait_split")
                    dinst = d.ins if hasattr(d, "ins") else d
                    for ob in blocks.values():
                        if ob.instructions and ob.instructions[-1] is dinst:
                            ob.instructions.pop()
                            break
                    dinst.sync_info = bass_rust.SyncInfo(on_wait=chunk, on_update=[])
                    bb.instructions.insert(i, dinst)
                    i += 1
            i += 1
    return nc


def _rne12(x):
    """Round fp32 mantissa to 11 explicit bits (RNE) — the float32r format."""
    v = np.ascontiguousarray(x, dtype=np.float32).view(np.uint32).astype(np.uint64)
    half = np.uint64(0x7FF) + ((v >> np.uint64(12)) & np.uint64(1))
    out = ((v + half) & np.uint64(0xFFFFF000)).astype(np.uint32)
    return out.view(np.float32)


def _build(reps=1, phases="A"):
    nc = bass.Bass()

    xt_in = nc.dram_tensor("xt", [FEAT, NTOT], BF16, kind="ExternalInput")
    xc_in = nc.dram_tensor("xc", [128, NTOT], F32R, kind="ExternalInput")
    wv_in = nc.dram_tensor("wv", [128, FEAT], BF16, kind="ExternalInput")
    w1t_in = nc.dram_tensor("w1t", [128, D], F32R, kind="ExternalInput")
    b1_in = nc.dram_tensor("b1", [D, 1], F32, kind="ExternalInput")
    w2dr_in = nc.dram_tensor("w2dr", [D, 2, N_SEQ], F8, kind="ExternalInput")
    wot_in = nc.dram_tensor("wot", [128, NC * FEAT], BF16, kind="ExternalInput")
    sel_in = nc.dram_tensor("sel", [2, 128], F32R, kind="ExternalInput")
    out_ext = nc.dram_tensor("out", [NBLK, FEAT], F32, kind="ExternalOutput")

    with tile.TileContext(nc) as tc, ExitStack() as ctx:
        wp = ctx.enter_context(tc.tile_pool(name="wp", bufs=1))
        dram = ctx.enter_context(tc.tile_pool(name="dram", bufs=1, space="DRAM"))

        # ---- resident weights/constants -------------------------------
        wv = wp.tile([128, FEAT], BF16)
        nc.sync.dma_start(wv[:], wv_in[:])
        w1t = wp.tile([128, D], F32R)           # W1.T stacked twice
        nc.sync.dma_start(w1t[:], w1t_in[:])
        b1t = wp.tile([D, 1], F32)
        nc.sync.dma_start(b1t[:], b1_in[:])
        w2dr = wp.tile([D, 2, N_SEQ], F8)       # 8*W2.T k-tiled; [0,1,:] = 8*b2
        nc.sync.dma_start(w2dr[:], w2dr_in[:])
        xc = wp.tile([128, NTOT], F32R)
        nc.sync.dma_start(xc[:], xc_in[:])
        sel = wp.tile([2, 128], F32R)
        nc.sync.dma_start(sel[:], sel_in[:])
        wot = wp.tile([128, NC * FEAT], BF16)

        vh = [wp.tile([128, MC, 130], BF16, name=f"vh{b}", tag=f"vh{b}") for b in range(B)]

        for _rep in range(reps):
            a2a_send = [dram.tile([NC, 65, NBLK], F32, name=f"snd{h}_{_rep}") for h in range(2)]
            a2a_recv = [dram.tile([NC, 65, NBLK], F32, name=f"rcv{h}_{_rep}") for h in range(2)]

            with ExitStack() as c2:
                # PSUM budget (8 banks):
                #   pvp: 2 tags x 2KB (hid ph + value accumulators)  = 2
                #   psl: 2 bufs x 4KB (double-wide DR logits)        = 4
                #   pso: 2 bufs x 2KB (S@V accumulators)             = 2
                pvp = c2.enter_context(tc.tile_pool(name="pvp", bufs=1, space="PSUM"))
                psl = c2.enter_context(tc.tile_pool(name="psl", bufs=2, space="PSUM"))
                pso = c2.enter_context(tc.tile_pool(name="pso", bufs=2, space="PSUM"))
                hp = c2.enter_context(tc.tile_pool(name="hp", bufs=2))
                ep = c2.enter_context(tc.tile_pool(name="ep", bufs=4))
                op = c2.enter_context(tc.tile_pool(name="op", bufs=4))

                # ---- hid MLP (only needs xc); fp8 k-tiled layout for the
                # DoubleRow logits matmul: k-tile 0 = relu(W1@x+b1),
                # k-tile 1 = [ones (fused b2) ; zeros] ------------------
                hidTs = []
                for h in range(2):
                    hidT = hp.tile([D, 2, NTOT], F8, name=f"hidT{h}", tag="hidT")
                    hidTs.append(hidT)
                    nc.vector.memset(hidT[:, 1, :], 0.0)
                    nc.vector.memset(hidT[0:1, 1, :], 1.0)
                    for nb in range(NB):
                        ph = pvp.tile([128, NBLK], F32, tag=f"pv{nb % 2}", name="ph")
                        nc.tensor.matmul(
                            ph[0:D, :],
                            w1t[h * D : (h + 1) * D, :],
                            xc[h * D : (h + 1) * D, nb * NBLK : (nb + 1) * NBLK],
                            start=True,
                            stop=True,
                            skip_group_check=True,
                        )
                        nc.scalar.activation(
                            hidT[:, 0, nb * NBLK : (nb + 1) * NBLK],
                            ph[0:D, :],
                            AF.Relu,
                            bias=b1t[:],
                        )

                # ---- P1: value projection directly in [m, d] layout:
                # per m-chunk, accumulate the 8 feature chunks in PSUM,
                # then split heads into vh (bf16) --------------------------
                for b in range(B):
                    nc.vector.memset(vh[b][:, :, 64:65], 1.0)
                    nc.vector.memset(vh[b][:, :, 129:130], 1.0)
                with tc.tile_pool(name="xfp", bufs=2) as xfp:
                    for g in range(2):  # g == batch
                        xfs = []
                        for f in range(8):
                            xf = xfp.tile([128, N_SEQ], BF16, tag=f"xf{f}", name=f"xf{f}")
                            nc.sync.dma_start(
                                xf[:],
                                xt_in[f * 128 : (f + 1) * 128, g * N_SEQ : (g + 1) * N_SEQ],
                            )
                            xfs.append(xf)
                        for j in range(MC):
                            pv = pvp.tile([128, 128], F32, tag=f"pv{j % 2}", name="pv")
                            for f in range(8):
                                nc.tensor.matmul(
                                    pv[:],
                                    xfs[f][:, j * 128 : (j + 1) * 128],
                                    wv[:, f * 128 : (f + 1) * 128],
                                    start=(f == 0),
                                    stop=(f == 7),
                                    skip_group_check=True,
                                )
                            nc.vector.tensor_copy(vh[g][:, j, 0:D], pv[:, 0:D])
                            nc.vector.tensor_copy(vh[g][:, j, 65 : 65 + D], pv[:, D:128])

                if _rep == 0:
                    nc.sync.dma_start(wot[:], wot_in[:])

                # ---- P2: attention ------------------------------------
                for h in range(2):
                    hidT = hidTs[h]
                    for nb in range(NB):
                        b = nb // (NB // B)
                        # quarter-size exp tiles (bufs=4): the pool slot
                        # barrier is per 4 m-chunks, not per block, so the
                        # next block's exp overlaps this block's S@V tail
                        eqs = []
                        for qt in range(4):
                            eq = ep.tile([128, 4 * NBLK], BF16, name="expTq", tag="expTq")
                            eqs.append(eq)
                            for jj in range(0, 4, 2):
                                pl = psl.tile([128, 2 * NBLK], F32)
                                for q in range(2):
                                    j = qt * 4 + jj + q
                                    nc.tensor.matmul(
                                        pl[:, q * NBLK : (q + 1) * NBLK],
                                        w2dr[:, :, j * 128 : (j + 1) * 128],
                                        hidT[:, :, nb * NBLK : (nb + 1) * NBLK],
                                        start=True,
                                        stop=True,
                                        perf_mode=mybir.MatmulPerfMode.DoubleRow,
                                        skip_group_check=True,
                                    )
                                if phases != "E":
                                    nc.scalar.activation(
                                        eq[:, jj * NBLK : (jj + 2) * NBLK],
                                        pl[:],
                                        AF.Exp,
                                        scale=1.0 / W2SCALE,
                                    )
                        po = pso.tile([65, NBLK], F32)
                        if phases != "X":
                            for j in range(MC):
                                nc.tensor.matmul(
                                    po[:],
                                    vh[b][:, j, h * 65 : (h + 1) * 65],
                                    eqs[j // 4][:, (j % 4) * NBLK : (j % 4 + 1) * NBLK],
                                    start=(j == 0),
                                    stop=(j == MC - 1),
                                    skip_group_check=True,
                                )
                        ot = op.tile([65, NBLK], F32)
                        nc.vector.tensor_copy(ot[:], po[:])
                        nc.sync.dma_start(a2a_send[h][nb], ot[:])

                    # fire this head's exchange as soon as its blocks are out
                    if phases not in ("1", "2"):
                        nc.gpsimd.collective_compute(
                            "AllToAll",
                            mybir.AluOpType.bypass,
                            ins=[a2a_send[h][:].opt()],
                            outs=[a2a_recv[h][:].opt()],
                            replica_groups=[list(range(NC))],
                        )

            if phases in ("1", "2", "3"):
                continue

            # ---- P4/P5: normalize + output projection -----------------
            with ExitStack() as c4:
                rp = c4.enter_context(tc.tile_pool(name="rp", bufs=6))
                psb = c4.enter_context(tc.tile_pool(name="psb", bufs=2, space="PSUM"))
                awp = c4.enter_context(tc.tile_pool(name="awp", bufs=1))
                psw = c4.enter_context(tc.tile_pool(name="psw", bufs=3, space="PSUM"))
                obp = c4.enter_context(tc.tile_pool(name="obp", bufs=3))

                actw = [awp.tile([128, NBLK], BF16, name=f"aw{s_}", tag=f"aw{s_}") for s_ in range(NC)]
                for s in range(NC):
                    sums = rp.tile([2, NBLK], F32)
                    nc.sync.dma_start(sums[0:1, :], a2a_recv[0][s, D : D + 1, :])
                    nc.sync.dma_start(sums[1:2, :], a2a_recv[1][s, D : D + 1, :])
                    raw = rp.tile([128, NBLK], F32)
                    nc.sync.dma_start(raw[0:D, :], a2a_recv[0][s, 0:D, :])
                    nc.sync.dma_start(raw[D:128, :], a2a_recv[1][s, 0:D, :])
                    rcps_f = rp.tile([2, NBLK], F32)
                    nc.vector.reciprocal(rcps_f[:], sums[:])
                    rcps = rp.tile([2, NBLK], F32R)
                    nc.vector.tensor_copy(rcps[:], rcps_f[:])
                    pb = psb.tile([128, NBLK], F32)
                    nc.tensor.matmul(
                        pb[:], sel[:], rcps[:], start=True, stop=True,
                        skip_group_check=True,
                    )
                    nc.vector.tensor_mul(actw[s][:], raw[:], pb[:])

                for t in range(NBLK // 128):
                    p0 = psw.tile([128, 512], F32, tag="pw0")
                    p1 = psw.tile([128, 512], F32, tag="pw1")
                    for s in range(NC):
                        nc.tensor.matmul(
                            p0[:],
                            actw[s][:, t * 128 : (t + 1) * 128],
                            wot[:, s * FEAT : s * FEAT + 512],
                            start=(s == 0),
                            stop=(s == NC - 1),
                            skip_group_check=True,
                        )
                        nc.tensor.matmul(
                            p1[:],
                            actw[s][:, t * 128 : (t + 1) * 128],
                            wot[:, s * FEAT + 512 : (s + 1) * FEAT],
                            start=(s == 0),
                            stop=(s == NC - 1),
                            skip_group_check=True,
                        )
                    ob = obp.tile([128, FEAT], F32)
                    nc.vector.tensor_copy(ob[:, 0:512], p0[:])
                    nc.vector.tensor_copy(ob[:, 512:1024], p1[:])
                    nc.sync.dma_start(out_ext[t * 128 : (t + 1) * 128, :], ob[:])

    _split_sem_waits(nc)
    return nc


_CACHE = {}


def _get_program(reps=1, phases="A"):
    key = ("nc", reps, phases)
    if key not in _CACHE:
        _CACHE[key] = _build(reps, phases)
    return _CACHE[key]


def kernel(x, W1, b1, W2, b2, Wv, Wo, _run_kwargs=None):
    x = np.asarray(x, dtype=np.float32)
    W1 = np.asarray(W1, dtype=np.float32)
    b1 = np.asarray(b1, dtype=np.float32)
    W2 = np.asarray(W2, dtype=np.float32)
    b2 = np.asarray(b2, dtype=np.float32)
    Wv = np.asarray(Wv, dtype=np.float32)
    Wo = np.asarray(Wo, dtype=np.float32)

    xr = x.reshape(NTOT, FEAT)
    xt_f = np.ascontiguousarray(xr.T)                          # [1024, 4096] f32
    xt_bf = xt_f.astype(ml_dtypes.bfloat16)
    xt_r = _rne12(xt_f)                                        # f32r for xc slices
    w1t = _rne12(np.concatenate([W1.T, W1.T], axis=0))         # [128, 64]
    # DoubleRow W2: k-tile 0 = 8*W2.T [64, 2048]; k-tile 1 row 0 = 8*b2
    w2dr = np.zeros((D, 2, N_SEQ), dtype=np.float32)
    w2dr[:, 0, :] = W2SCALE * W2.T
    w2dr[0, 1, :] = W2SCALE * b2
    w2dr = w2dr.astype(ml_dtypes.float8_e4m3)
    wot = (
        Wo.T.reshape(NC, 128, FEAT).transpose(1, 0, 2).reshape(128, NC * FEAT)
    ).astype(ml_dtypes.bfloat16)
    b1c = np.ascontiguousarray(b1.reshape(D, 1))
    sel_h = np.zeros((2, 128), dtype=np.float32)
    sel_h[0, :D] = 1.0
    sel_h[1, D:] = 1.0

    in_maps = []
    for c in range(NC):
        wv_c_blocks = Wv[c * 128 : (c + 1) * 128, :]           # [128 d, 1024 f]
        wv_c = np.concatenate(
            [wv_c_blocks[:, f * 128 : (f + 1) * 128].T for f in range(8)], axis=1
        ).astype(ml_dtypes.bfloat16)                           # [128 f, 1024] col-block f
        in_maps.append(
            {
                "xt": xt_bf,
                "xc": np.ascontiguousarray(xt_r[c * 128 : (c + 1) * 128, :]),
                "wv": wv_c,
                "w1t": w1t,
                "b1": b1c,
                "w2dr": w2dr,
                "wot": wot,
                "sel": sel_h,
            }
        )

    import os
    nc = _get_program(
        int(os.environ.get("KERNEL_REPS", "1")), os.environ.get("KERNEL_PHASES", "A")
    )
    res = run_bass_kernel_spmd(
        nc, in_maps, list(range(NC)), **(_run_kwargs or {})
    )
    out = np.concatenate([res.results[c]["out"] for c in range(NC)], axis=0)
    if _run_kwargs:
        kernel.last_results = res
    return out.reshape(B, N_SEQ, FEAT)


# revision 4
# speedup vs baseline: 1.6792x; 1.6792x over previous
"""MultiHeadDenseAttention on 8 Trainium2 NeuronCores.

Head-sharded tensor parallelism: each core computes 2 of 16 heads
(value projection slice, per-head MLP attention logits, softmax, S@V),
then an AllToAll exchanges head-blocks for row-blocks so each core
computes the output projection for its 512 rows with the full Wo.

v2 vs v1: mixed-precision row-count reduction on the PE.
  - logits matmul in fp8e4m3 DoubleRow mode (2 k-tiles of 64 contracted
    per pass, 0.5 cycles/row): hid stored fp8 [64, 2, 4096] with k-tile 1
    holding the ones row (fused b2) + zero padding; W2 host-scaled by 8
    (fp8 dynamic range) and un-scaled in the exp activation (scale=1/8).
  - value projection computed directly in [m, d] layout (lhsT = x chunk,
    rhs = Wv.T block, PSUM accumulation over the 8 feature chunks):
    kills the 32 fp32 PE transposes and the SBUF re-accumulation pass.
  - value/S@V/output-projection in bf16 (same PE rate as fp32r at these
    sizes, half the SBUF/DMA bytes); x streamed as bf16 (8.4 MB/rep).

Layouts (per core c, heads 2c / 2c+1):
  xt    [1024, 4096] bf16  x.reshape(4096,1024).T  (feat on partitions)
  xc    [128, 4096] f32r   xt rows [128c, 128c+128) (this core's head cols)
  wv    [128, 1024] bf16   col block f = Wv.T[128f:+128, 128c:+128]
  vh[b] [128, 16, 130] bf16  per m-chunk: 65 (h0 dims+one) + 65 (h1)
  hidT[h] [64, 2, 4096] fp8  k-tile 0 = relu(W1@x), k-tile 1 = ones/zeros
  w2dr  [64, 2, 2048] fp8  k-tile 0 = 8*W2.T, k-tile 1 row 0 = 8*b2
  logitsT/expT [128m, 512n] per m-chunk (softmax w/o max: logits O(1))
  S@V: po[65, 512] = vh_aug.T @ expT  (row 64 = sum of exp)
  A2A [8, 2, 65, 512] f32, normalize after exchange, out = act @ Wo.T.
"""

import sys

if "/opt/trn_rl_repo" not in sys.path:
    sys.path.insert(0, "/opt/trn_rl_repo")

from contextlib import ExitStack

import numpy as np
import ml_dtypes

import bass_rust
import concourse.bass as bass
import concourse.tile as tile
from concourse import mybir
from concourse.bass_utils import run_bass_kernel_spmd

F32 = mybir.dt.float32
F32R = mybir.dt.float32r
BF16 = mybir.dt.bfloat16
F8 = mybir.dt.float8e4
AF = mybir.ActivationFunctionType

NC = 8            # cores
B = 2             # batch
N_SEQ = 2048      # seq len == max_seq_len (m)
FEAT = 1024
H = 16            # heads
D = 64            # head dim
NTOT = B * N_SEQ  # 4096 flattened rows
NBLK = 512        # n-block size
NB = NTOT // NBLK # 8 n-blocks (== A2A shards == cores)
MC = N_SEQ // 128 # 16 m-chunks per batch
W2SCALE = 8.0     # host-side W2 scaling into fp8 range; undone in exp


def _split_sem_waits(nc, limit=1):
    """Walrus rejects instructions with more than ~1 sync wait; move the
    excess onto NOPs on the same engine inserted immediately before."""
    blocks = {}
    for f in nc.m.functions:
        for bb in f.blocks:
            blocks[bb.name] = bb
    for bb in blocks.values():
        i = 0
        while i < len(bb.instructions):
            inst = bb.instructions[i]
            si = inst.sync_info
            if si is not None and si.on_wait and len(si.on_wait) > limit:
                waits = list(si.on_wait)
                chunks = [waits[j : j + limit] for j in range(0, len(waits), limit)]
                si.on_wait = chunks[-1]
                engine = nc.engines[inst.engine]
                for chunk in chunks[:-1]:
                    d = engine.nop(nofuse=True, hint="wait_split")
                    dinst = d.ins if hasattr(d, "ins") else d
                    for ob in blocks.values():
                        if ob.instructions and ob.instructions[-1] is dinst:
                            ob.instructions.pop()
                            break
                    dinst.sync_info = bass_rust.SyncInfo(on_wait=chunk, on_update=[])
                    bb.instructions.insert(i, dinst)
                    i += 1
            i += 1
    return nc


def _rne12(x):
    """Round fp32 mantissa to 11 explicit bits (RNE) — the float32r format."""
    v = np.ascontiguousarray(x, dtype=np.float32).view(np.uint32).astype(np.uint64)
    half = np.uint64(0x7FF) + ((v >> np.uint64(12)) & np.uint64(1))
    out = ((v + half) & np.uint64(0xFFFFF000)).astype(np.uint32)
    return out.view(np.float32)


def _build(reps=1, phases="A"):
    nc = bass.Bass()

    xt_in = nc.dram_tensor("xt", [FEAT, NTOT], BF16, kind="ExternalInput")
    xc_in = nc.dram_tensor("xc", [128, NTOT], F32R, kind="ExternalInput")
    wv_in = nc.dram_tensor("wv", [128, FEAT], BF16, kind="ExternalInput")
    w1t_in = nc.dram_tensor("w1t", [128, D], F32R, kind="ExternalInput")
    b1_in = nc.dram_tensor("b1", [D, 1], F32, kind="ExternalInput")
    w2dr_in = nc.dram_tensor("w2dr", [D, 2, N_SEQ], F8, kind="ExternalInput")
    wot_in = nc.dram_tensor("wot", [128, NC * FEAT], BF16, kind="ExternalInput")
    sel_in = nc.dram_tensor("sel", [2, 128], F32R, kind="ExternalInput")
    out_ext = nc.dram_tensor("out", [NBLK, FEAT], F32, kind="ExternalOutput")

    with tile.TileContext(nc) as tc, ExitStack() as ctx:
        wp = ctx.enter_context(tc.tile_pool(name="wp", bufs=1))
        dram = ctx.enter_context(tc.tile_pool(name="dram", bufs=1, space="DRAM"))

        # ---- resident weights/constants -------------------------------
        wv = wp.tile([128, FEAT], BF16)
        nc.sync.dma_start(wv[:], wv_in[:])
        w1t = wp.tile([128, D], F32R)           # W1.T stacked twice
        nc.sync.dma_start(w1t[:], w1t_in[:])
        b1t = wp.tile([D, 1], F32)
        nc.sync.dma_start(b1t[:], b1_in[:])
        w2dr = wp.tile([D, 2, N_SEQ], F8)       # 8*W2.T k-tiled; [0,1,:] = 8*b2
        nc.sync.dma_start(w2dr[:], w2dr_in[:])
        xc = wp.tile([128, NTOT], F32R)
        nc.sync.dma_start(xc[:], xc_in[:])
        sel = wp.tile([2, 128], F32R)
        nc.sync.dma_start(sel[:], sel_in[:])
        wot = wp.tile([128, NC * FEAT], BF16)

        vh = [wp.tile([128, MC, 130], BF16, name=f"vh{b}", tag=f"vh{b}") for b in range(B)]

        for _rep in range(reps):
            a2a_send = [dram.tile([NC, 65, NBLK], F32, name=f"snd{h}_{_rep}") for h in range(2)]
            a2a_recv = [dram.tile([NC, 65, NBLK], F32, name=f"rcv{h}_{_rep}") for h in range(2)]

            with ExitStack() as c2:
                # PSUM budget (8 banks):
                #   pvp: 2 tags x 2KB (hid ph + value accumulators)  = 2
                #   psl: 2 bufs x 4KB (double-wide DR logits)        = 4
                #   pso: 2 bufs x 2KB (S@V accumulators)             = 2
                pvp = c2.enter_context(tc.tile_pool(name="pvp", bufs=1, space="PSUM"))
                psl = c2.enter_context(tc.tile_pool(name="psl", bufs=2, space="PSUM"))
                pso = c2.enter_context(tc.tile_pool(name="pso", bufs=2, space="PSUM"))
                hp = c2.enter_context(tc.tile_pool(name="hp", bufs=2))
                ep = c2.enter_context(tc.tile_pool(name="ep", bufs=4))
                op = c2.enter_context(tc.tile_pool(name="op", bufs=4))

                # ---- hid MLP (only needs xc); fp8 k-tiled layout for the
                # DoubleRow logits matmul: k-tile 0 = relu(W1@x+b1),
                # k-tile 1 = [ones (fused b2) ; zeros] ------------------
                hidTs = []
                for h in range(2):
                    hidT = hp.tile([D, 2, NTOT], F8, name=f"hidT{h}", tag="hidT")
                    hidTs.append(hidT)
                    nc.vector.memset(hidT[:, 1, :], 0.0)
                    nc.vector.memset(hidT[0:1, 1, :], 1.0)
                    for nb in range(NB):
                        ph = pvp.tile([128, NBLK], F32, tag=f"pv{nb % 2}", name="ph")
                        nc.tensor.matmul(
                            ph[0:D, :],
                            w1t[h * D : (h + 1) * D, :],
                            xc[h * D : (h + 1) * D, nb * NBLK : (nb + 1) * NBLK],
                            start=True,
                            stop=True,
                            skip_group_check=True,
                        )
                        nc.scalar.activation(
                            hidT[:, 0, nb * NBLK : (nb + 1) * NBLK],
                            ph[0:D, :],
                            AF.Relu,
                            bias=b1t[:],
                        )

                # ---- P1: value projection directly in [m, d] layout:
                # per m-chunk, accumulate the 8 feature chunks in PSUM,
                # then split heads into vh (bf16) --------------------------
                for b in range(B):
                    nc.vector.memset(vh[b][:, :, 64:65], 1.0)
                    nc.vector.memset(vh[b][:, :, 129:130], 1.0)
                with tc.tile_pool(name="xfp", bufs=2) as xfp:
                    for g in range(2):  # g == batch
                        xfs = []
                        for f in range(8):
                            xf = xfp.tile([128, N_SEQ], BF16, tag=f"xf{f}", name=f"xf{f}")
                            nc.sync.dma_start(
                                xf[:],
                                xt_in[f * 128 : (f + 1) * 128, g * N_SEQ : (g + 1) * N_SEQ],
                            )
                            xfs.append(xf)
                        for j in range(MC):
                            pv = pvp.tile([128, 128], F32, tag=f"pv{j % 2}", name="pv")
                            for f in range(8):
                                nc.tensor.matmul(
                                    pv[:],
                                    xfs[f][:, j * 128 : (j + 1) * 128],
                                    wv[:, f * 128 : (f + 1) * 128],
                                    start=(f == 0),
                                    stop=(f == 7),
                                    skip_group_check=True,
                                )
                            nc.vector.tensor_copy(vh[g][:, j, 0:D], pv[:, 0:D])
                            nc.vector.tensor_copy(vh[g][:, j, 65 : 65 + D], pv[:, D:128])

                if _rep == 0:
                    nc.sync.dma_start(wot[:], wot_in[:])

                # ---- P2: attention ------------------------------------
                for h in range(2):
                    hidT = hidTs[h]
                    for nb in range(NB):
                        b = nb // (NB // B)
                        # quarter-size exp tiles (bufs=4): the pool slot
                        # barrier is per 4 m-chunks, not per block, so the
                        # next block's exp overlaps this block's S@V tail
                        eqs = []
                        for qt in range(4):
                            eq = ep.tile([128, 4 * NBLK], BF16, name="expTq", tag="expTq")
                            eqs.append(eq)
                            if phases == "E":
                                nc.vector.memset(eq[:, :], 1.0)
                            for jj in range(0, 4, 2):
                                pl = psl.tile([128, 2 * NBLK], F32)
                                for q in range(2):
                                    j = qt * 4 + jj + q
                                    nc.tensor.matmul(
                                        pl[:, q * NBLK : (q + 1) * NBLK],
                                        w2dr[:, :, j * 128 : (j + 1) * 128],
                                        hidT[:, :, nb * NBLK : (nb + 1) * NBLK],
                                        start=True,
                                        stop=True,
                                        perf_mode=mybir.MatmulPerfMode.DoubleRow,
                                        skip_group_check=True,
                                    )
                                if phases != "E":
                                    nc.scalar.activation(
                                        eq[:, jj * NBLK : (jj + 2) * NBLK],
                                        pl[:],
                                        AF.Exp,
                                        scale=1.0 / W2SCALE,
                                    )
                        po = pso.tile([65, NBLK], F32)
                        if phases != "X":
                            for j in range(MC):
                                nc.tensor.matmul(
                                    po[:],
                                    vh[b][:, j, h * 65 : (h + 1) * 65],
                                    eqs[j // 4][:, (j % 4) * NBLK : (j % 4 + 1) * NBLK],
                                    start=(j == 0),
                                    stop=(j == MC - 1),
                                    skip_group_check=True,
                                )
                        ot = op.tile([65, NBLK], F32)
                        nc.vector.tensor_copy(ot[:], po[:])
                        nc.sync.dma_start(a2a_send[h][nb], ot[:])

                    # fire this head's exchange as soon as its blocks are out
                    if phases not in ("1", "2"):
                        nc.gpsimd.collective_compute(
                            "AllToAll",
                            mybir.AluOpType.bypass,
                            ins=[a2a_send[h][:].opt()],
                            outs=[a2a_recv[h][:].opt()],
                            replica_groups=[list(range(NC))],
                        )

            if phases in ("1", "2", "3"):
                continue

            # ---- P4/P5: normalize + output projection -----------------
            with ExitStack() as c4:
                rp = c4.enter_context(tc.tile_pool(name="rp", bufs=6))
                psb = c4.enter_context(tc.tile_pool(name="psb", bufs=2, space="PSUM"))
                awp = c4.enter_context(tc.tile_pool(name="awp", bufs=1))
                psw = c4.enter_context(tc.tile_pool(name="psw", bufs=3, space="PSUM"))
                obp = c4.enter_context(tc.tile_pool(name="obp", bufs=3))

                actw = [awp.tile([128, NBLK], BF16, name=f"aw{s_}", tag=f"aw{s_}") for s_ in range(NC)]
                for s in range(NC):
                    sums = rp.tile([2, NBLK], F32)
                    nc.sync.dma_start(sums[0:1, :], a2a_recv[0][s, D : D + 1, :])
                    nc.sync.dma_start(sums[1:2, :], a2a_recv[1][s, D : D + 1, :])
                    raw = rp.tile([128, NBLK], F32)
                    nc.sync.dma_start(raw[0:D, :], a2a_recv[0][s, 0:D, :])
                    nc.sync.dma_start(raw[D:128, :], a2a_recv[1][s, 0:D, :])
                    rcps_f = rp.tile([2, NBLK], F32)
                    nc.vector.reciprocal(rcps_f[:], sums[:])
                    rcps = rp.tile([2, NBLK], F32R)
                    nc.vector.tensor_copy(rcps[:], rcps_f[:])
                    pb = psb.tile([128, NBLK], F32)
                    nc.tensor.matmul(
                        pb[:], sel[:], rcps[:], start=True, stop=True,
                        skip_group_check=True,
                    )
                    nc.vector.tensor_mul(actw[s][:], raw[:], pb[:])

                for t in range(NBLK // 128):
                    p0 = psw.tile([128, 512], F32, tag="pw0")
                    p1 = psw.tile([128, 512], F32, tag="pw1")
                    for s in range(NC):
                        nc.tensor.matmul(
                            p0[:],
                            actw[s][:, t * 128 : (t + 1) * 128],
                            wot[:, s * FEAT : s * FEAT + 512],
                            start=(s == 0),
                            stop=(s == NC - 1),
                            skip_group_check=True,
                        )
                        nc.tensor.matmul(
                            p1[:],
                            actw[s][:, t * 128 : (t + 1) * 128],
                            wot[:, s * FEAT + 512 : (s + 1) * FEAT],
                            start=(s == 0),
                            stop=(s == NC - 1),
                            skip_group_check=True,
                        )
                    ob = obp.tile([128, FEAT], F32)
                    nc.vector.tensor_copy(ob[:, 0:512], p0[:])
                    nc.vector.tensor_copy(ob[:, 512:1024], p1[:])
                    nc.sync.dma_start(out_ext[t * 128 : (t + 1) * 128, :], ob[:])

    _split_sem_waits(nc)
    return nc


_CACHE = {}


def _get_program(reps=1, phases="A"):
    key = ("nc", reps, phases)
    if key not in _CACHE:
        _CACHE[key] = _build(reps, phases)
    return _CACHE[key]


def kernel(x, W1, b1, W2, b2, Wv, Wo, _run_kwargs=None):
    x = np.asarray(x, dtype=np.float32)
    W1 = np.asarray(W1, dtype=np.float32)
    b1 = np.asarray(b1, dtype=np.float32)
    W2 = np.asarray(W2, dtype=np.float32)
    b2 = np.asarray(b2, dtype=np.float32)
    Wv = np.asarray(Wv, dtype=np.float32)
    Wo = np.asarray(Wo, dtype=np.float32)

    xr = x.reshape(NTOT, FEAT)
    xt_f = np.ascontiguousarray(xr.T)                          # [1024, 4096] f32
    xt_bf = xt_f.astype(ml_dtypes.bfloat16)
    xt_r = _rne12(xt_f)                                        # f32r for xc slices
    w1t = _rne12(np.concatenate([W1.T, W1.T], axis=0))         # [128, 64]
    # DoubleRow W2: k-tile 0 = 8*W2.T [64, 2048]; k-tile 1 row 0 = 8*b2
    w2dr = np.zeros((D, 2, N_SEQ), dtype=np.float32)
    w2dr[:, 0, :] = W2SCALE * W2.T
    w2dr[0, 1, :] = W2SCALE * b2
    w2dr = w2dr.astype(ml_dtypes.float8_e4m3)
    wot = (
        Wo.T.reshape(NC, 128, FEAT).transpose(1, 0, 2).reshape(128, NC * FEAT)
    ).astype(ml_dtypes.bfloat16)
    b1c = np.ascontiguousarray(b1.reshape(D, 1))
    sel_h = np.zeros((2, 128), dtype=np.float32)
    sel_h[0, :D] = 1.0
    sel_h[1, D:] = 1.0

    in_maps = []
    for c in range(NC):
        wv_c_blocks = Wv[c * 128 : (c + 1) * 128, :]           # [128 d, 1024 f]
        wv_c = np.concatenate(
            [wv_c_blocks[:, f * 128 : (f + 1) * 128].T for f in range(8)], axis=1
        ).astype(ml_dtypes.bfloat16)                           # [128 f, 1024] col-block f
        in_maps.append(
            {
                "xt": xt_bf,
                "xc": np.ascontiguousarray(xt_r[c * 128 : (c + 1) * 128, :]),
                "wv": wv_c,
                "w1t": w1t,
                "b1": b1c,
                "w2dr": w2dr,
                "wot": wot,
                "sel": sel_h,
            }
        )

    import os
    nc = _get_program(
        int(os.environ.get("KERNEL_REPS", "1")), os.environ.get("KERNEL_PHASES", "A")
    )
    res = run_bass_kernel_spmd(
        nc, in_maps, list(range(NC)), **(_run_kwargs or {})
    )
    out = np.concatenate([res.results[c]["out"] for c in range(NC)], axis=0)
    if _run_kwargs:
        kernel.last_results = res
    return out.reshape(B, N_SEQ, FEAT)


# revision 5
# speedup vs baseline: 2.7141x; 1.6163x over previous
"""MultiHeadDenseAttention on 8 Trainium2 NeuronCores.

Head-sharded tensor parallelism: each core computes 2 of 16 heads
(value projection slice, per-head MLP attention logits, softmax, S@V),
then an AllToAll exchanges head-blocks for row-blocks so each core
computes the output projection for its 512 rows with the full Wo.

v2 vs v1: mixed-precision row-count reduction on the PE.
  - logits matmul in fp8e4m3 DoubleRow mode (2 k-tiles of 64 contracted
    per pass, 0.5 cycles/row): hid stored fp8 [64, 2, 4096] with k-tile 1
    holding the ones row (fused b2) + zero padding; W2 host-scaled by 8
    (fp8 dynamic range) and un-scaled in the exp activation (scale=1/8).
  - value projection computed directly in [m, d] layout (lhsT = x chunk,
    rhs = Wv.T block, PSUM accumulation over the 8 feature chunks):
    kills the 32 fp32 PE transposes and the SBUF re-accumulation pass.
  - value/S@V/output-projection in bf16 (same PE rate as fp32r at these
    sizes, half the SBUF/DMA bytes); x streamed as bf16 (8.4 MB/rep).

Layouts (per core c, heads 2c / 2c+1):
  xt    [1024, 4096] bf16  x.reshape(4096,1024).T  (feat on partitions)
  xc    [128, 4096] f32r   xt rows [128c, 128c+128) (this core's head cols)
  wv    [128, 1024] bf16   col block f = Wv.T[128f:+128, 128c:+128]
  vh[b] [128, 16, 130] bf16  per m-chunk: 65 (h0 dims+one) + 65 (h1)
  hidT[h] [64, 2, 4096] fp8  k-tile 0 = relu(W1@x), k-tile 1 = ones/zeros
  w2dr  [64, 2, 2048] fp8  k-tile 0 = 8*W2.T, k-tile 1 row 0 = 8*b2
  logitsT/expT [128m, 512n] per m-chunk (softmax w/o max: logits O(1))
  S@V: po[65, 512] = vh_aug.T @ expT  (row 64 = sum of exp)
  A2A [8, 2, 65, 512] f32, normalize after exchange, out = act @ Wo.T.
"""

import sys

if "/opt/trn_rl_repo" not in sys.path:
    sys.path.insert(0, "/opt/trn_rl_repo")

from contextlib import ExitStack

import numpy as np
import ml_dtypes

import bass_rust
import concourse.bass as bass
import concourse.tile as tile
from concourse import mybir
from concourse.bass_utils import run_bass_kernel_spmd

F32 = mybir.dt.float32
F32R = mybir.dt.float32r
BF16 = mybir.dt.bfloat16
F8 = mybir.dt.float8e4
AF = mybir.ActivationFunctionType

NC = 8            # cores
B = 2             # batch
N_SEQ = 2048      # seq len == max_seq_len (m)
FEAT = 1024
H = 16            # heads
D = 64            # head dim
NTOT = B * N_SEQ  # 4096 flattened rows
NBLK = 512        # n-block size
NB = NTOT // NBLK # 8 n-blocks (== A2A shards == cores)
MC = N_SEQ // 128 # 16 m-chunks per batch
W2SCALE = 8.0     # host-side W2 scaling into fp8 range; undone in exp


def _split_sem_waits(nc, limit=1):
    """Walrus rejects instructions with more than ~1 sync wait; move the
    excess onto NOPs on the same engine inserted immediately before."""
    blocks = {}
    for f in nc.m.functions:
        for bb in f.blocks:
            blocks[bb.name] = bb
    for bb in blocks.values():
        i = 0
        while i < len(bb.instructions):
            inst = bb.instructions[i]
            si = inst.sync_info
            if si is not None and si.on_wait and len(si.on_wait) > limit:
                waits = list(si.on_wait)
                chunks = [waits[j : j + limit] for j in range(0, len(waits), limit)]
                si.on_wait = chunks[-1]
                engine = nc.engines[inst.engine]
                for chunk in chunks[:-1]:
                    d = engine.nop(nofuse=True, hint="wait_split")
                    dinst = d.ins if hasattr(d, "ins") else d
                    for ob in blocks.values():
                        if ob.instructions and ob.instructions[-1] is dinst:
                            ob.instructions.pop()
                            break
                    dinst.sync_info = bass_rust.SyncInfo(on_wait=chunk, on_update=[])
                    bb.instructions.insert(i, dinst)
                    i += 1
            i += 1
    return nc


def _rne12(x):
    """Round fp32 mantissa to 11 explicit bits (RNE) — the float32r format."""
    v = np.ascontiguousarray(x, dtype=np.float32).view(np.uint32).astype(np.uint64)
    half = np.uint64(0x7FF) + ((v >> np.uint64(12)) & np.uint64(1))
    out = ((v + half) & np.uint64(0xFFFFF000)).astype(np.uint32)
    return out.view(np.float32)


def _build(reps=1, phases="A"):
    nc = bass.Bass()

    xt_in = nc.dram_tensor("xt", [FEAT, NTOT], BF16, kind="ExternalInput")
    xc_in = nc.dram_tensor("xc", [128, NTOT], F32R, kind="ExternalInput")
    wv_in = nc.dram_tensor("wv", [128, FEAT], BF16, kind="ExternalInput")
    w1t_in = nc.dram_tensor("w1t", [128, D], F32R, kind="ExternalInput")
    b1_in = nc.dram_tensor("b1", [D, 1], F32, kind="ExternalInput")
    w2dr_in = nc.dram_tensor("w2dr", [D, 2, N_SEQ], F8, kind="ExternalInput")
    wot_in = nc.dram_tensor("wot", [128, NC * FEAT], BF16, kind="ExternalInput")
    sel_in = nc.dram_tensor("sel", [2, 128], F32R, kind="ExternalInput")
    out_ext = nc.dram_tensor("out", [NBLK, FEAT], F32, kind="ExternalOutput")

    with tile.TileContext(nc) as tc, ExitStack() as ctx:
        wp = ctx.enter_context(tc.tile_pool(name="wp", bufs=1))
        dram = ctx.enter_context(tc.tile_pool(name="dram", bufs=1, space="DRAM"))

        # ---- resident weights/constants -------------------------------
        wv = wp.tile([128, FEAT], BF16)
        nc.sync.dma_start(wv[:], wv_in[:])
        w1t = wp.tile([128, D], F32R)           # W1.T stacked twice
        nc.sync.dma_start(w1t[:], w1t_in[:])
        b1t = wp.tile([D, 1], F32)
        nc.sync.dma_start(b1t[:], b1_in[:])
        w2dr = wp.tile([D, 2, N_SEQ], F8)       # 8*W2.T k-tiled; [0,1,:] = 8*b2
        nc.sync.dma_start(w2dr[:], w2dr_in[:])
        xc = wp.tile([128, NTOT], F32R)
        nc.sync.dma_start(xc[:], xc_in[:])
        sel = wp.tile([2, 128], F32R)
        nc.sync.dma_start(sel[:], sel_in[:])
        wot = wp.tile([128, NC * FEAT], BF16)

        vh = [wp.tile([128, MC, 130], BF16, name=f"vh{b}", tag=f"vh{b}") for b in range(B)]

        for _rep in range(reps):
            a2a_send = [dram.tile([NC, 65, NBLK], F32, name=f"snd{h}_{_rep}") for h in range(2)]
            a2a_recv = [dram.tile([NC, 65, NBLK], F32, name=f"rcv{h}_{_rep}") for h in range(2)]

            with ExitStack() as c2:
                # PSUM budget (8 banks):
                #   pvp: 2 tags x 2KB (hid ph + value accumulators)  = 2
                #   psl: 2 bufs x 4KB (double-wide DR logits)        = 4
                #   pso: 2 bufs x 2KB (S@V accumulators)             = 2
                pvp = c2.enter_context(tc.tile_pool(name="pvp", bufs=1, space="PSUM"))
                psl = c2.enter_context(tc.tile_pool(name="psl", bufs=2, space="PSUM"))
                pso = c2.enter_context(tc.tile_pool(name="pso", bufs=2, space="PSUM"))
                hp = c2.enter_context(tc.tile_pool(name="hp", bufs=2))
                ep = c2.enter_context(tc.tile_pool(name="ep", bufs=4))
                op = c2.enter_context(tc.tile_pool(name="op", bufs=4))

                # ---- hid MLP (only needs xc); fp8 k-tiled layout for the
                # DoubleRow logits matmul: k-tile 0 = relu(W1@x+b1),
                # k-tile 1 = [ones (fused b2) ; zeros] ------------------
                hidTs = []
                for h in range(2):
                    hidT = hp.tile([D, 2, NTOT], F8, name=f"hidT{h}", tag="hidT")
                    hidTs.append(hidT)
                    nc.vector.memset(hidT[:, 1, :], 0.0)
                    nc.vector.memset(hidT[0:1, 1, :], 1.0)
                    for nb in range(NB):
                        ph = pvp.tile([128, NBLK], F32, tag=f"pv{nb % 2}", name="ph")
                        nc.tensor.matmul(
                            ph[0:D, :],
                            w1t[h * D : (h + 1) * D, :],
                            xc[h * D : (h + 1) * D, nb * NBLK : (nb + 1) * NBLK],
                            start=True,
                            stop=True,
                            skip_group_check=True,
                        )
                        nc.scalar.activation(
                            hidT[:, 0, nb * NBLK : (nb + 1) * NBLK],
                            ph[0:D, :],
                            AF.Relu,
                            bias=b1t[:],
                        )

                # ---- P1: value projection directly in [m, d] layout:
                # per m-chunk, accumulate the 8 feature chunks in PSUM,
                # then split heads into vh (bf16) --------------------------
                for b in range(B):
                    nc.vector.memset(vh[b][:, :, 64:65], 1.0)
                    nc.vector.memset(vh[b][:, :, 129:130], 1.0)
                with tc.tile_pool(name="xfp", bufs=2) as xfp:
                    for g in range(2):  # g == batch
                        xfs = []
                        for f in range(8):
                            xf = xfp.tile([128, N_SEQ], BF16, tag=f"xf{f}", name=f"xf{f}")
                            nc.sync.dma_start(
                                xf[:],
                                xt_in[f * 128 : (f + 1) * 128, g * N_SEQ : (g + 1) * N_SEQ],
                            )
                            xfs.append(xf)
                        for j in range(MC):
                            pv = pvp.tile([128, 128], F32, tag=f"pv{j % 2}", name="pv")
                            for f in range(8):
                                nc.tensor.matmul(
                                    pv[:],
                                    xfs[f][:, j * 128 : (j + 1) * 128],
                                    wv[:, f * 128 : (f + 1) * 128],
                                    start=(f == 0),
                                    stop=(f == 7),
                                    skip_group_check=True,
                                )
                            nc.vector.tensor_copy(vh[g][:, j, 0:D], pv[:, 0:D])
                            nc.vector.tensor_copy(vh[g][:, j, 65 : 65 + D], pv[:, D:128])

                if _rep == 0:
                    nc.sync.dma_start(wot[:], wot_in[:])

                # ---- P2: attention ------------------------------------
                for h in range(2):
                    hidT = hidTs[h]
                    for nb in range(NB):
                        b = nb // (NB // B)
                        # quarter-size exp tiles (bufs=4): the pool slot
                        # barrier is per 4 m-chunks, not per block, so the
                        # next block's exp overlaps this block's S@V tail
                        eqs = []
                        for qt in range(4):
                            eq = ep.tile([128, 4 * NBLK], BF16, name="expTq", tag="expTq")
                            eqs.append(eq)
                            if phases == "E":
                                nc.vector.memset(eq[:, :], 1.0)
                            for jj in range(0, 4, 2):
                                pl = psl.tile([128, 2 * NBLK], F32)
                                for q in range(2):
                                    j = qt * 4 + jj + q
                                    nc.tensor.matmul(
                                        pl[:, q * NBLK : (q + 1) * NBLK],
                                        w2dr[:, :, j * 128 : (j + 1) * 128],
                                        hidT[:, :, nb * NBLK : (nb + 1) * NBLK],
                                        start=True,
                                        stop=True,
                                        perf_mode=mybir.MatmulPerfMode.DoubleRow,
                                        skip_group_check=True,
                                    )
                                if phases != "E":
                                    nc.scalar.activation(
                                        eq[:, jj * NBLK : (jj + 2) * NBLK],
                                        pl[:],
                                        AF.Exp,
                                        scale=1.0 / W2SCALE,
                                    )
                        po = pso.tile([65, NBLK], F32)
                        if phases == "X":
                            nc.vector.memset(po[:], 1.0)
                        else:
                            for j in range(MC):
                                nc.tensor.matmul(
                                    po[:],
                                    vh[b][:, j, h * 65 : (h + 1) * 65],
                                    eqs[j // 4][:, (j % 4) * NBLK : (j % 4 + 1) * NBLK],
                                    start=(j == 0),
                                    stop=(j == MC - 1),
                                    skip_group_check=True,
                                )
                        ot = op.tile([65, NBLK], F32)
                        nc.vector.tensor_copy(ot[:], po[:])
                        nc.sync.dma_start(a2a_send[h][nb], ot[:])

                    # fire this head's exchange as soon as its blocks are out
                    if phases not in ("1", "2"):
                        nc.gpsimd.collective_compute(
                            "AllToAll",
                            mybir.AluOpType.bypass,
                            ins=[a2a_send[h][:].opt()],
                            outs=[a2a_recv[h][:].opt()],
                            replica_groups=[list(range(NC))],
                        )

            if phases in ("1", "2", "3"):
                continue

            # ---- P4/P5: normalize + output projection -----------------
            with ExitStack() as c4:
                rp = c4.enter_context(tc.tile_pool(name="rp", bufs=6))
                psb = c4.enter_context(tc.tile_pool(name="psb", bufs=2, space="PSUM"))
                awp = c4.enter_context(tc.tile_pool(name="awp", bufs=1))
                psw = c4.enter_context(tc.tile_pool(name="psw", bufs=3, space="PSUM"))
                obp = c4.enter_context(tc.tile_pool(name="obp", bufs=3))

                actw = [awp.tile([128, NBLK], BF16, name=f"aw{s_}", tag=f"aw{s_}") for s_ in range(NC)]
                for s in range(NC):
                    sums = rp.tile([2, NBLK], F32)
                    nc.sync.dma_start(sums[0:1, :], a2a_recv[0][s, D : D + 1, :])
                    nc.sync.dma_start(sums[1:2, :], a2a_recv[1][s, D : D + 1, :])
                    raw = rp.tile([128, NBLK], F32)
                    nc.sync.dma_start(raw[0:D, :], a2a_recv[0][s, 0:D, :])
                    nc.sync.dma_start(raw[D:128, :], a2a_recv[1][s, 0:D, :])
                    rcps_f = rp.tile([2, NBLK], F32)
                    nc.vector.reciprocal(rcps_f[:], sums[:])
                    rcps = rp.tile([2, NBLK], F32R)
                    nc.vector.tensor_copy(rcps[:], rcps_f[:])
                    pb = psb.tile([128, NBLK], F32)
                    nc.tensor.matmul(
                        pb[:], sel[:], rcps[:], start=True, stop=True,
                        skip_group_check=True,
                    )
                    nc.vector.tensor_mul(actw[s][:], raw[:], pb[:])

                for t in range(NBLK // 128):
                    p0 = psw.tile([128, 512], F32, tag="pw0")
                    p1 = psw.tile([128, 512], F32, tag="pw1")
                    for s in range(NC):
                        nc.tensor.matmul(
                            p0[:],
                            actw[s][:, t * 128 : (t + 1) * 128],
                            wot[:, s * FEAT : s * FEAT + 512],
                            start=(s == 0),
                            stop=(s == NC - 1),
                            skip_group_check=True,
                        )
                        nc.tensor.matmul(
                            p1[:],
                            actw[s][:, t * 128 : (t + 1) * 128],
                            wot[:, s * FEAT + 512 : (s + 1) * FEAT],
                            start=(s == 0),
                            stop=(s == NC - 1),
                            skip_group_check=True,
                        )
                    ob = obp.tile([128, FEAT], F32)
                    nc.vector.tensor_copy(ob[:, 0:512], p0[:])
                    nc.vector.tensor_copy(ob[:, 512:1024], p1[:])
                    nc.sync.dma_start(out_ext[t * 128 : (t + 1) * 128, :], ob[:])

    _split_sem_waits(nc)
    return nc


_CACHE = {}


def _get_program(reps=1, phases="A"):
    key = ("nc", reps, phases)
    if key not in _CACHE:
        _CACHE[key] = _build(reps, phases)
    return _CACHE[key]


def kernel(x, W1, b1, W2, b2, Wv, Wo, _run_kwargs=None):
    x = np.asarray(x, dtype=np.float32)
    W1 = np.asarray(W1, dtype=np.float32)
    b1 = np.asarray(b1, dtype=np.float32)
    W2 = np.asarray(W2, dtype=np.float32)
    b2 = np.asarray(b2, dtype=np.float32)
    Wv = np.asarray(Wv, dtype=np.float32)
    Wo = np.asarray(Wo, dtype=np.float32)

    xr = x.reshape(NTOT, FEAT)
    xt_f = np.ascontiguousarray(xr.T)                          # [1024, 4096] f32
    xt_bf = xt_f.astype(ml_dtypes.bfloat16)
    xt_r = _rne12(xt_f)                                        # f32r for xc slices
    w1t = _rne12(np.concatenate([W1.T, W1.T], axis=0))         # [128, 64]
    # DoubleRow W2: k-tile 0 = 8*W2.T [64, 2048]; k-tile 1 row 0 = 8*b2
    w2dr = np.zeros((D, 2, N_SEQ), dtype=np.float32)
    w2dr[:, 0, :] = W2SCALE * W2.T
    w2dr[0, 1, :] = W2SCALE * b2
    w2dr = w2dr.astype(ml_dtypes.float8_e4m3)
    wot = (
        Wo.T.reshape(NC, 128, FEAT).transpose(1, 0, 2).reshape(128, NC * FEAT)
    ).astype(ml_dtypes.bfloat16)
    b1c = np.ascontiguousarray(b1.reshape(D, 1))
    sel_h = np.zeros((2, 128), dtype=np.float32)
    sel_h[0, :D] = 1.0
    sel_h[1, D:] = 1.0

    in_maps = []
    for c in range(NC):
        wv_c_blocks = Wv[c * 128 : (c + 1) * 128, :]           # [128 d, 1024 f]
        wv_c = np.concatenate(
            [wv_c_blocks[:, f * 128 : (f + 1) * 128].T for f in range(8)], axis=1
        ).astype(ml_dtypes.bfloat16)                           # [128 f, 1024] col-block f
        in_maps.append(
            {
                "xt": xt_bf,
                "xc": np.ascontiguousarray(xt_r[c * 128 : (c + 1) * 128, :]),
                "wv": wv_c,
                "w1t": w1t,
                "b1": b1c,
                "w2dr": w2dr,
                "wot": wot,
                "sel": sel_h,
            }
        )

    import os
    nc = _get_program(
        int(os.environ.get("KERNEL_REPS", "1")), os.environ.get("KERNEL_PHASES", "A")
    )
    res = run_bass_kernel_spmd(
        nc, in_maps, list(range(NC)), **(_run_kwargs or {})
    )
    out = np.concatenate([res.results[c]["out"] for c in range(NC)], axis=0)
    if _run_kwargs:
        kernel.last_results = res
    return out.reshape(B, N_SEQ, FEAT)


# revision 6
# speedup vs baseline: 12.0379x; 4.4354x over previous
"""MultiHeadDenseAttention on 8 Trainium2 NeuronCores.

Head-sharded tensor parallelism: each core computes 2 of 16 heads
(value projection slice, per-head MLP attention logits, softmax, S@V),
then an AllToAll exchanges head-blocks for row-blocks so each core
computes the output projection for its 512 rows with the full Wo.

v2 vs v1: mixed-precision row-count reduction on the PE.
  - logits matmul in fp8e4m3 DoubleRow mode (2 k-tiles of 64 contracted
    per pass, 0.5 cycles/row): hid stored fp8 [64, 2, 4096] with k-tile 1
    holding the ones row (fused b2) + zero padding; W2 host-scaled by 8
    (fp8 dynamic range) and un-scaled in the exp activation (scale=1/8).
  - value projection computed directly in [m, d] layout (lhsT = x chunk,
    rhs = Wv.T block, PSUM accumulation over the 8 feature chunks):
    kills the 32 fp32 PE transposes and the SBUF re-accumulation pass.
  - value/S@V/output-projection in bf16 (same PE rate as fp32r at these
    sizes, half the SBUF/DMA bytes); x streamed as bf16 (8.4 MB/rep).

Layouts (per core c, heads 2c / 2c+1):
  xt    [1024, 4096] bf16  x.reshape(4096,1024).T  (feat on partitions)
  xc    [128, 4096] f32r   xt rows [128c, 128c+128) (this core's head cols)
  wv    [128, 1024] bf16   col block f = Wv.T[128f:+128, 128c:+128]
  vh[b] [128, 16, 130] bf16  per m-chunk: 65 (h0 dims+one) + 65 (h1)
  hidT[h] [64, 2, 4096] fp8  k-tile 0 = relu(W1@x), k-tile 1 = ones/zeros
  w2dr  [64, 2, 2048] fp8  k-tile 0 = 8*W2.T, k-tile 1 row 0 = 8*b2
  logitsT/expT [128m, 512n] per m-chunk (softmax w/o max: logits O(1))
  S@V: po[65, 512] = vh_aug.T @ expT  (row 64 = sum of exp)
  A2A [8, 2, 65, 512] f32, normalize after exchange, out = act @ Wo.T.
"""

import sys

if "/opt/trn_rl_repo" not in sys.path:
    sys.path.insert(0, "/opt/trn_rl_repo")

from contextlib import ExitStack

import numpy as np
import ml_dtypes

import bass_rust
import concourse.bass as bass
import concourse.tile as tile
from concourse import mybir
from concourse.bass_utils import run_bass_kernel_spmd

F32 = mybir.dt.float32
F32R = mybir.dt.float32r
BF16 = mybir.dt.bfloat16
F8 = mybir.dt.float8e4
AF = mybir.ActivationFunctionType

NC = 8            # cores
B = 2             # batch
N_SEQ = 2048      # seq len == max_seq_len (m)
FEAT = 1024
H = 16            # heads
D = 64            # head dim
NTOT = B * N_SEQ  # 4096 flattened rows
NBLK = 512        # n-block size
NB = NTOT // NBLK # 8 n-blocks (== A2A shards == cores)
MC = N_SEQ // 128 # 16 m-chunks per batch
W2SCALE = 8.0     # host-side W2 scaling into fp8 range; undone in exp


def _split_sem_waits(nc, limit=1):
    """Walrus rejects instructions with more than ~1 sync wait; move the
    excess onto NOPs on the same engine inserted immediately before."""
    blocks = {}
    for f in nc.m.functions:
        for bb in f.blocks:
            blocks[bb.name] = bb
    for bb in blocks.values():
        i = 0
        while i < len(bb.instructions):
            inst = bb.instructions[i]
            si = inst.sync_info
            if si is not None and si.on_wait and len(si.on_wait) > limit:
                waits = list(si.on_wait)
                chunks = [waits[j : j + limit] for j in range(0, len(waits), limit)]
                si.on_wait = chunks[-1]
                engine = nc.engines[inst.engine]
                for chunk in chunks[:-1]:
                    d = engine.nop(nofuse=True, hint="wait_split")
                    dinst = d.ins if hasattr(d, "ins") else d
                    for ob in blocks.values():
                        if ob.instructions and ob.instructions[-1] is dinst:
                            ob.instructions.pop()
                            break
                    dinst.sync_info = bass_rust.SyncInfo(on_wait=chunk, on_update=[])
                    bb.instructions.insert(i, dinst)
                    i += 1
            i += 1
    return nc


def _rne12(x):
    """Round fp32 mantissa to 11 explicit bits (RNE) — the float32r format."""
    v = np.ascontiguousarray(x, dtype=np.float32).view(np.uint32).astype(np.uint64)
    half = np.uint64(0x7FF) + ((v >> np.uint64(12)) & np.uint64(1))
    out = ((v + half) & np.uint64(0xFFFFF000)).astype(np.uint32)
    return out.view(np.float32)


def _build(reps=1, phases="A"):
    nc = bass.Bass()

    xt_in = nc.dram_tensor("xt", [FEAT, NTOT], BF16, kind="ExternalInput")
    xc_in = nc.dram_tensor("xc", [128, NTOT], F32R, kind="ExternalInput")
    wv_in = nc.dram_tensor("wv", [128, FEAT], BF16, kind="ExternalInput")
    w1t_in = nc.dram_tensor("w1t", [128, D], F32R, kind="ExternalInput")
    b1_in = nc.dram_tensor("b1", [D, 1], F32, kind="ExternalInput")
    w2dr_in = nc.dram_tensor("w2dr", [D, 2, N_SEQ], F8, kind="ExternalInput")
    wot_in = nc.dram_tensor("wot", [128, NC * FEAT], BF16, kind="ExternalInput")
    sel_in = nc.dram_tensor("sel", [2, 128], F32R, kind="ExternalInput")
    out_ext = nc.dram_tensor("out", [NBLK, FEAT], F32, kind="ExternalOutput")

    with tile.TileContext(nc) as tc, ExitStack() as ctx:
        wp = ctx.enter_context(tc.tile_pool(name="wp", bufs=1))
        dram = ctx.enter_context(tc.tile_pool(name="dram", bufs=1, space="DRAM"))

        # ---- resident weights/constants -------------------------------
        wv = wp.tile([128, FEAT], BF16)
        nc.sync.dma_start(wv[:], wv_in[:])
        w1t = wp.tile([128, D], F32R)           # W1.T stacked twice
        nc.sync.dma_start(w1t[:], w1t_in[:])
        b1t = wp.tile([D, 1], F32)
        nc.sync.dma_start(b1t[:], b1_in[:])
        w2dr = wp.tile([D, 2, N_SEQ], F8)       # 8*W2.T k-tiled; [0,1,:] = 8*b2
        nc.sync.dma_start(w2dr[:], w2dr_in[:])
        xc = wp.tile([128, NTOT], F32R)
        nc.sync.dma_start(xc[:], xc_in[:])
        sel = wp.tile([2, 128], F32R)
        nc.sync.dma_start(sel[:], sel_in[:])
        wot = wp.tile([128, NC * FEAT], BF16)

        vh = [wp.tile([128, MC, 132], BF16, name=f"vh{b}", tag=f"vh{b}") for b in range(B)]

        for _rep in range(reps):
            a2a_send = [dram.tile([NC, 65, NBLK], F32, name=f"snd{h}_{_rep}") for h in range(2)]
            a2a_recv = [dram.tile([NC, 65, NBLK], F32, name=f"rcv{h}_{_rep}") for h in range(2)]

            with ExitStack() as c2:
                # PSUM budget (8 banks):
                #   pvp: 2 tags x 2KB (hid ph + value accumulators)  = 2
                #   psl: 2 bufs x 4KB (double-wide DR logits)        = 4
                #   pso: 2 bufs x 2KB (S@V accumulators)             = 2
                pvp = c2.enter_context(tc.tile_pool(name="pvp", bufs=1, space="PSUM"))
                psl = c2.enter_context(tc.tile_pool(name="psl", bufs=2, space="PSUM"))
                pso = c2.enter_context(tc.tile_pool(name="pso", bufs=2, space="PSUM"))
                hp = c2.enter_context(tc.tile_pool(name="hp", bufs=2))
                ep = c2.enter_context(tc.tile_pool(name="ep", bufs=4))
                op = c2.enter_context(tc.tile_pool(name="op", bufs=4))

                # ---- hid MLP (only needs xc); fp8 k-tiled layout for the
                # DoubleRow logits matmul: k-tile 0 = relu(W1@x+b1),
                # k-tile 1 = [ones (fused b2) ; zeros] ------------------
                hidTs = []
                for h in range(2):
                    hidT = hp.tile([D, 2, NTOT], F8, name=f"hidT{h}", tag="hidT")
                    hidTs.append(hidT)
                    nc.vector.memset(hidT[:, 1, :], 0.0)
                    nc.vector.memset(hidT[0:1, 1, :], 1.0)
                    for nb in range(NB):
                        ph = pvp.tile([128, NBLK], F32, tag=f"pv{nb % 2}", name="ph")
                        nc.tensor.matmul(
                            ph[0:D, :],
                            w1t[h * D : (h + 1) * D, :],
                            xc[h * D : (h + 1) * D, nb * NBLK : (nb + 1) * NBLK],
                            start=True,
                            stop=True,
                            skip_group_check=True,
                        )
                        nc.scalar.activation(
                            hidT[:, 0, nb * NBLK : (nb + 1) * NBLK],
                            ph[0:D, :],
                            AF.Relu,
                            bias=b1t[:],
                        )

                # ---- P1: value projection directly in [m, d] layout:
                # per m-chunk, accumulate the 8 feature chunks in PSUM,
                # then split heads into vh (bf16) --------------------------
                for b in range(B):
                    nc.vector.memset(vh[b][:, :, 64:65], 1.0)
                    nc.vector.memset(vh[b][:, :, 130:131], 1.0)
                with tc.tile_pool(name="xfp", bufs=2) as xfp:
                    for g in range(2):  # g == batch
                        xfs = []
                        for f in range(8):
                            xf = xfp.tile([128, N_SEQ], BF16, tag=f"xf{f}", name=f"xf{f}")
                            nc.sync.dma_start(
                                xf[:],
                                xt_in[f * 128 : (f + 1) * 128, g * N_SEQ : (g + 1) * N_SEQ],
                            )
                            xfs.append(xf)
                        for j in range(MC):
                            pv = pvp.tile([128, 128], F32, tag=f"pv{j % 2}", name="pv")
                            for f in range(8):
                                nc.tensor.matmul(
                                    pv[:],
                                    xfs[f][:, j * 128 : (j + 1) * 128],
                                    wv[:, f * 128 : (f + 1) * 128],
                                    start=(f == 0),
                                    stop=(f == 7),
                                    skip_group_check=True,
                                )
                            nc.vector.tensor_copy(vh[g][:, j, 0:D], pv[:, 0:D])
                            nc.vector.tensor_copy(vh[g][:, j, 66 : 66 + D], pv[:, D:128])

                if _rep == 0:
                    nc.sync.dma_start(wot[:], wot_in[:])

                # ---- P2: attention ------------------------------------
                for h in range(2):
                    hidT = hidTs[h]
                    for nb in range(NB):
                        b = nb // (NB // B)
                        # quarter-size exp tiles (bufs=4): the pool slot
                        # barrier is per 4 m-chunks, not per block, so the
                        # next block's exp overlaps this block's S@V tail
                        eqs = []
                        for qt in range(4):
                            eq = ep.tile([128, 4 * NBLK], BF16, name="expTq", tag="expTq")
                            eqs.append(eq)
                            if phases == "E":
                                nc.vector.memset(eq[:, :], 1.0)
                            for jj in range(0, 4, 2):
                                pl = psl.tile([128, 2 * NBLK], F32)
                                for q in range(2):
                                    j = qt * 4 + jj + q
                                    nc.tensor.matmul(
                                        pl[:, q * NBLK : (q + 1) * NBLK],
                                        w2dr[:, :, j * 128 : (j + 1) * 128],
                                        hidT[:, :, nb * NBLK : (nb + 1) * NBLK],
                                        start=True,
                                        stop=True,
                                        perf_mode=mybir.MatmulPerfMode.DoubleRow,
                                        skip_group_check=True,
                                    )
                                if phases != "E":
                                    nc.scalar.activation(
                                        eq[:, jj * NBLK : (jj + 2) * NBLK],
                                        pl[:],
                                        AF.Exp,
                                        scale=1.0 / W2SCALE,
                                    )
                        po = pso.tile([65, NBLK], F32)
                        if phases == "X":
                            nc.vector.memset(po[:], 1.0)
                        else:
                            for j in range(MC):
                                nc.tensor.matmul(
                                    po[:],
                                    vh[b][:, j, h * 66 : h * 66 + 65],
                                    eqs[j // 4][:, (j % 4) * NBLK : (j % 4 + 1) * NBLK],
                                    start=(j == 0),
                                    stop=(j == MC - 1),
                                    skip_group_check=True,
                                )
                        ot = op.tile([65, NBLK], F32)
                        nc.vector.tensor_copy(ot[:], po[:])
                        nc.sync.dma_start(a2a_send[h][nb], ot[:])

                    # fire this head's exchange as soon as its blocks are out
                    if phases not in ("1", "2"):
                        nc.gpsimd.collective_compute(
                            "AllToAll",
                            mybir.AluOpType.bypass,
                            ins=[a2a_send[h][:].opt()],
                            outs=[a2a_recv[h][:].opt()],
                            replica_groups=[list(range(NC))],
                        )

            if phases in ("1", "2", "3"):
                continue

            # ---- P4/P5: normalize + output projection -----------------
            with ExitStack() as c4:
                rp = c4.enter_context(tc.tile_pool(name="rp", bufs=6))
                psb = c4.enter_context(tc.tile_pool(name="psb", bufs=2, space="PSUM"))
                awp = c4.enter_context(tc.tile_pool(name="awp", bufs=1))
                psw = c4.enter_context(tc.tile_pool(name="psw", bufs=3, space="PSUM"))
                obp = c4.enter_context(tc.tile_pool(name="obp", bufs=3))

                actw = [awp.tile([128, NBLK], BF16, name=f"aw{s_}", tag=f"aw{s_}") for s_ in range(NC)]
                for s in range(NC):
                    sums = rp.tile([2, NBLK], F32)
                    nc.sync.dma_start(sums[0:1, :], a2a_recv[0][s, D : D + 1, :])
                    nc.sync.dma_start(sums[1:2, :], a2a_recv[1][s, D : D + 1, :])
                    raw = rp.tile([128, NBLK], F32)
                    nc.sync.dma_start(raw[0:D, :], a2a_recv[0][s, 0:D, :])
                    nc.sync.dma_start(raw[D:128, :], a2a_recv[1][s, 0:D, :])
                    rcps_f = rp.tile([2, NBLK], F32)
                    nc.vector.reciprocal(rcps_f[:], sums[:])
                    rcps = rp.tile([2, NBLK], F32R)
                    nc.vector.tensor_copy(rcps[:], rcps_f[:])
                    pb = psb.tile([128, NBLK], F32)
                    nc.tensor.matmul(
                        pb[:], sel[:], rcps[:], start=True, stop=True,
                        skip_group_check=True,
                    )
                    nc.vector.tensor_mul(actw[s][:], raw[:], pb[:])

                for t in range(NBLK // 128):
                    p0 = psw.tile([128, 512], F32, tag="pw0")
                    p1 = psw.tile([128, 512], F32, tag="pw1")
                    for s in range(NC):
                        nc.tensor.matmul(
                            p0[:],
                            actw[s][:, t * 128 : (t + 1) * 128],
                            wot[:, s * FEAT : s * FEAT + 512],
                            start=(s == 0),
                            stop=(s == NC - 1),
                            skip_group_check=True,
                        )
                        nc.tensor.matmul(
                            p1[:],
                            actw[s][:, t * 128 : (t + 1) * 128],
                            wot[:, s * FEAT + 512 : (s + 1) * FEAT],
                            start=(s == 0),
                            stop=(s == NC - 1),
                            skip_group_check=True,
                        )
                    ob = obp.tile([128, FEAT], F32)
                    nc.vector.tensor_copy(ob[:, 0:512], p0[:])
                    nc.vector.tensor_copy(ob[:, 512:1024], p1[:])
                    nc.sync.dma_start(out_ext[t * 128 : (t + 1) * 128, :], ob[:])

    _split_sem_waits(nc)
    return nc


_CACHE = {}


def _get_program(reps=1, phases="A"):
    key = ("nc", reps, phases)
    if key not in _CACHE:
        _CACHE[key] = _build(reps, phases)
    return _CACHE[key]


def kernel(x, W1, b1, W2, b2, Wv, Wo, _run_kwargs=None):
    x = np.asarray(x, dtype=np.float32)
    W1 = np.asarray(W1, dtype=np.float32)
    b1 = np.asarray(b1, dtype=np.float32)
    W2 = np.asarray(W2, dtype=np.float32)
    b2 = np.asarray(b2, dtype=np.float32)
    Wv = np.asarray(Wv, dtype=np.float32)
    Wo = np.asarray(Wo, dtype=np.float32)

    xr = x.reshape(NTOT, FEAT)
    xt_f = np.ascontiguousarray(xr.T)                          # [1024, 4096] f32
    xt_bf = xt_f.astype(ml_dtypes.bfloat16)
    xt_r = _rne12(xt_f)                                        # f32r for xc slices
    w1t = _rne12(np.concatenate([W1.T, W1.T], axis=0))         # [128, 64]
    # DoubleRow W2: k-tile 0 = 8*W2.T [64, 2048]; k-tile 1 row 0 = 8*b2
    w2dr = np.zeros((D, 2, N_SEQ), dtype=np.float32)
    w2dr[:, 0, :] = W2SCALE * W2.T
    w2dr[0, 1, :] = W2SCALE * b2
    w2dr = w2dr.astype(ml_dtypes.float8_e4m3)
    wot = (
        Wo.T.reshape(NC, 128, FEAT).transpose(1, 0, 2).reshape(128, NC * FEAT)
    ).astype(ml_dtypes.bfloat16)
    b1c = np.ascontiguousarray(b1.reshape(D, 1))
    sel_h = np.zeros((2, 128), dtype=np.float32)
    sel_h[0, :D] = 1.0
    sel_h[1, D:] = 1.0

    in_maps = []
    for c in range(NC):
        wv_c_blocks = Wv[c * 128 : (c + 1) * 128, :]           # [128 d, 1024 f]
        wv_c = np.concatenate(
            [wv_c_blocks[:, f * 128 : (f + 1) * 128].T for f in range(8)], axis=1
        ).astype(ml_dtypes.bfloat16)                           # [128 f, 1024] col-block f
        in_maps.append(
            {
                "xt": xt_bf,
                "xc": np.ascontiguousarray(xt_r[c * 128 : (c + 1) * 128, :]),
                "wv": wv_c,
                "w1t": w1t,
                "b1": b1c,
                "w2dr": w2dr,
                "wot": wot,
                "sel": sel_h,
            }
        )

    import os
    nc = _get_program(
        int(os.environ.get("KERNEL_REPS", "1")), os.environ.get("KERNEL_PHASES", "A")
    )
    res = run_bass_kernel_spmd(
        nc, in_maps, list(range(NC)), **(_run_kwargs or {})
    )
    out = np.concatenate([res.results[c]["out"] for c in range(NC)], axis=0)
    if _run_kwargs:
        kernel.last_results = res
    return out.reshape(B, N_SEQ, FEAT)
